# revision 3
# baseline (speedup 1.0000x reference)
"""Multi-head attention (B=4, T=S=2048, E=1024, H=16) on 8 trn2 NeuronCores.

Sharding: core c handles batch b = c // 2 and head-half hh = c % 2
(8 of 16 heads).  Host sums the two partial outputs per batch, adds bo.

v2 on-chip design (vs v1):
  - scores.T = kp @ qp.T per (head-pair j, t-block tb, s-chunk) as before,
    exp on ACT -> pt [s, t] bf16.
  - PV reoriented: ctx[t, d] with t on psum partitions: lhsT = pt slice
    (K=s 128, M=t 128), rhs = vp_ext (N=65: 64 dims + ones column for the
    softmax denominator).  Halves PE rows for PV, and the denominator
    lands on a column indexed by partition t -> normalize is a cheap DVE
    reciprocal + per-partition tensor_scalar_mul (no broadcast matmuls).
  - normalized ctx is PE-transposed back to [e, t] to feed the output
    projection; out psum is DMA'd straight to DRAM (no sbuf bounce).
  - emission is streamed: k-proj(pair 0) / q-proj(pair0, tb0) / first v
    chunks gate the first attention unit; all remaining loads/transposes/
    projections are interleaved into the attention units' slots, with
    load->transpose->consume adjacency per stream so every buffer-ring
    reuse happens after all consumers of the evicted tile are emitted.
"""

import numpy as np

import concourse.bass as bass
import concourse.mybir as mybir
import concourse.tile as tile
from concourse.bass_utils import run_bass_kernel_spmd
from concourse.masks import make_identity

F32 = mybir.dt.float32
BF16 = mybir.dt.bfloat16

B, T, E = 4, 2048, 1024
H = 16  # global heads
HL = 8  # heads per core (local)
HD = 64  # head dim
EL = HL * HD  # 512, e-dims per core
N_CORES = 8

_CACHED = {}


def legalize_waits(nc, cap=1):
    """Hoist semaphore waits so no instruction carries more than `cap`.
    (single wait slot in this container's walrus; see v1)"""
    import bass_rust

    totals = {}
    names = {}
    for f in nc.m.functions:
        for bb in f.blocks:
            for ins in bb.instructions:
                si = ins.sync_info
                if si is None:
                    continue
                for u in si.on_update or []:
                    if u.sync_type == "semaphore":
                        sign = 1 if u.update_mode in ("sem-inc", "sem-add-imm") else -1
                        totals[u.id] = totals.get(u.id, 0) + sign * u.update_value
                        names[u.id] = u.ant_name

    n = 0
    for f in nc.m.functions:
        for bb in f.blocks:
            insts = bb.instructions
            out = []
            changed = False
            for ins in insts:
                if type(ins).__name__ == "InstISA" and "RANGE_CLEAR" in str(ins):
                    import re

                    m = re.search(r"range_first=(\d+) range_last=(\d+)", str(ins))
                    first, last = int(m.group(1)), int(m.group(2))
                    for sid in range(first, last + 1):
                        tot = totals.get(sid, 0)
                        if tot == 0:
                            continue
                        ev = mybir.InstEventSemaphore(name=f"I-LC{n}", ins=[], outs=[])
                        n += 1
                        ev.engine = ins.engine
                        ev.sync_info = bass_rust.SyncInfo(
                            on_wait=[],
                            on_update=[
                                bass_rust.SyncUpdate(
                                    sync_type="semaphore",
                                    id=sid,
                                    ant_name=names.get(sid, f"sem{sid}"),
                                    update_mode="sem-sub-imm",
                                    update_value=tot,
                                    update_reg=None,
                                )
                            ],
                        )
                        out.append(ev)
                    changed = True
                    continue
                si = ins.sync_info
                ws = list(si.on_wait) if (si is not None and si.on_wait) else []
                if len(ws) > cap:
                    for w in ws[: len(ws) - cap]:
                        ev = mybir.InstEventSemaphore(
                            name=f"I-LW{n}", ins=[], outs=[]
                        )
                        n += 1
                        ev.engine = ins.engine
                        ev.sync_info = bass_rust.SyncInfo(
                            on_wait=[w], on_update=[]
                        )
                        out.append(ev)
                    si.on_wait = ws[len(ws) - cap :]
                    changed = True
                out.append(ins)
            if changed:
                insts[:] = out
    return n


def build_program():
    nc = bass.Bass()

    qd = nc.declare_dram_parameter("q", [T, E], F32, isOutput=False)
    kd = nc.declare_dram_parameter("k", [T, E], F32, isOutput=False)
    vd = nc.declare_dram_parameter("v", [T, E], F32, isOutput=False)
    wqd = nc.declare_dram_parameter("wq", [EL, E], F32, isOutput=False)
    wkd = nc.declare_dram_parameter("wk", [EL, E], F32, isOutput=False)
    wvd = nc.declare_dram_parameter("wv", [EL, E], F32, isOutput=False)
    wod = nc.declare_dram_parameter("wo", [E, EL], F32, isOutput=False)
    bqd = nc.declare_dram_parameter("bq", [EL], F32, isOutput=False)
    bkd = nc.declare_dram_parameter("bk", [EL], F32, isOutput=False)
    bvd = nc.declare_dram_parameter("bv", [EL], F32, isOutput=False)
    outd = nc.declare_dram_parameter("outT", [E, T], BF16, isOutput=True)

    with tile.TileContext(nc, pool_alloc_mode="queue") as tc:
        with (
            tc.tile_pool(name="singles", bufs=1) as singles,
            tc.tile_pool(name="stage", bufs=2) as stage,
            tc.tile_pool(name="xtr", bufs=2) as xtrp,
            tc.tile_pool(name="acts", bufs=1) as acts,
            tc.tile_pool(name="pt", bufs=4) as ptp,
            tc.tile_pool(name="ctxn", bufs=2) as ctxnp,
            tc.tile_pool(name="norm", bufs=2) as normp,
            tc.tile_pool(name="sc_ps", bufs=2, space="PSUM") as sc_ps,
            tc.tile_pool(name="ctx_ps", bufs=2, space="PSUM") as ctx_ps,
            tc.tile_pool(name="mp_ps", bufs=2, space="PSUM") as mp_ps,
        ):
            # ---------------- consts -------------------------------------
            ident = singles.tile([128, 128], BF16)
            make_identity(nc, ident)
            ones_col = singles.tile([1, 128], BF16)
            nc.vector.memset(ones_col, 1.0)
            zero_row = singles.tile([1, 512], BF16)
            nc.vector.memset(zero_row, 0.0)

            bq_sb = singles.tile([128, 4], F32)
            bk_sb = singles.tile([128, 4], F32)
            nc.gpsimd.dma_start(out=bq_sb, in_=bqd.rearrange("(c p) -> p c", p=128))
            nc.gpsimd.dma_start(out=bk_sb, in_=bkd.rearrange("(c p) -> p c", p=128))
            bv_sb = singles.tile([1, EL], BF16)
            nc.gpsimd.dma_start(out=bv_sb, in_=bvd.rearrange("(o e) -> o e", o=1))

            wqT = singles.tile([128, 8, EL], BF16)
            wkT = singles.tile([128, 8, EL], BF16)
            wvT = singles.tile([128, 8, EL], BF16)
            woT = singles.tile([128, 4, E], BF16)

            def load_nat(xd, r0, nrows, tag="nat", bufs=2):
                """rows [r0, r0+nrows) of f32 DRAM -> bf16 natural SBUF
                [128, nrows//128, ncols] (cast in the SWDGE DMA)."""
                a = nrows // 128
                ncols = xd.shape[1]
                xb = stage.tile([128, a, ncols], BF16, tag=tag, bufs=bufs)
                step = min(4, a)
                for blk in range(0, a, step):
                    nc.gpsimd.dma_start(
                        out=xb[:, blk : blk + step, :],
                        in_=xd[r0 + blk * 128 : r0 + (blk + step) * 128, :].rearrange(
                            "(a p) e -> p a e", p=128
                        ),
                    )
                return xb

            def pe_tr(dst_fn, src, a_chunks, e_chunks):
                """dst(e)[p, a*128+t] = src[t(p), a, e*128+p] via PE
                transposes through the sc psum ring (prologue only)."""
                for e in range(e_chunks):
                    n = a_chunks * 128
                    trt = sc_ps.tile([128, n], BF16, tag="sc")
                    for a in range(a_chunks):
                        nc.tensor.transpose(
                            trt[:, a * 128 : (a + 1) * 128],
                            src[:, a, e * 128 : (e + 1) * 128],
                            ident,
                        )
                    nc.vector.tensor_copy(out=dst_fn(e), in_=trt)

            def dma_tr(dst, dst_off, nat, n_chunks):
                """xbar transpose-DMA: natural [128, c, 1024] -> transposed
                dst[:, :, dst_off + i*128 ...] (mapping f = c*128+p, verified
                == the PE-transpose layout)."""
                for i in range(n_chunks):
                    nc.sync.dma_start_transpose(
                        out=dst[:, :, dst_off + i * 128 : dst_off + (i + 1) * 128],
                        in_=nat[:, i, :],
                    )

            # ---------------- activations --------------------------------
            qpT = acts.tile([128, 4, T], BF16)
            kpT = acts.tile([128, 4, T], BF16)
            vp_ext = acts.tile([128, 16, HL * 65], BF16)
            ctxnT = acts.tile([128, 4, T], BF16)
            xTq = acts.tile([128, 8, T], BF16)  # q transposed, resident

            def kqproj(xT_blk, wT, b_sb, xpT, c, blk):
                ps = mp_ps.tile([128, 512], F32, tag="mp")
                for e in range(8):
                    nc.tensor.matmul(
                        ps,
                        lhsT=wT[:, e, c * 128 : (c + 1) * 128],
                        rhs=xT_blk[:, e, :],
                        start=(e == 0),
                        stop=(e == 7),
                    )
                nc.vector.tensor_scalar_add(
                    out=xpT[:, c, blk * 512 : (blk + 1) * 512],
                    in0=ps,
                    scalar1=b_sb[:, c : c + 1],
                )

            def qproj(c, tb):
                ps = mp_ps.tile([128, 512], F32, tag="mp")
                for e in range(8):
                    nc.tensor.matmul(
                        ps,
                        lhsT=wqT[:, e, c * 128 : (c + 1) * 128],
                        rhs=xTq[:, e, tb * 512 : (tb + 1) * 512],
                        start=(e == 0),
                        stop=(e == 7),
                    )
                nc.vector.tensor_scalar_add(
                    out=qpT[:, c, tb * 512 : (tb + 1) * 512],
                    in0=ps,
                    scalar1=bq_sb[:, c : c + 1],
                )

            def vproj(xT_blk, s):
                """vp_ext[:, s, :] from v-transposed block s//4."""
                off = (s % 4) * 128
                ps = mp_ps.tile([128, 512], F32, tag="mp")
                for e in range(8):
                    nc.tensor.matmul(
                        ps,
                        lhsT=xT_blk[:, e, off : off + 128],
                        rhs=wvT[:, e, :],
                        start=(e == 0),
                        stop=False,
                    )
                nc.tensor.matmul(ps, lhsT=ones_col, rhs=bv_sb, start=False, stop=True)
                nc.vector.memset(vp_ext[:, s, :], 1.0)
                nc.vector.tensor_copy(
                    out=vp_ext[:, s, :].rearrange("p (h x) -> p h x", x=65)[:, :, 0:64],
                    in_=ps.rearrange("p (h d) -> p h d", d=64),
                )

            def xload_dmatr(xd, blk, tag):
                """load 512-row block blk, xbar-transpose to [128, 8, 512]."""
                nat = load_nat(xd, blk * 512, 512)
                xt = xtrp.tile([128, 8, 512], BF16, tag=tag, bufs=(4 if tag == "xk" else 2))
                dma_tr(xt, 0, nat, 4)
                return xt

            # ---------------- prologue -----------------------------------
            # PE transposes for everything latency-critical (xbar transpose
            # DMAs hold the single-slot HWDGE ~1us per 128-row chunk, so
            # they only run where DMA is otherwise idle: wo + k re-loads).
            def xload_pe(xd, blk, tag):
                nat = load_nat(xd, blk * 512, 512)
                xt = xtrp.tile([128, 8, 512], BF16, tag=tag, bufs=(4 if tag == "xk" else 2))
                pe_tr(lambda e: xt[:, e, :], nat, a_chunks=4, e_chunks=8)
                return xt

            wkn = load_nat(wkd, 0, EL)
            pe_tr(lambda e: wkT[:, e, :], wkn, a_chunks=4, e_chunks=8)
            kT0 = xload_pe(kd, 0, "xk")
            kqproj(kT0, wkT, bk_sb, kpT, 0, 0)

            wqn = load_nat(wqd, 0, EL)
            pe_tr(lambda e: wqT[:, e, :], wqn, a_chunks=4, e_chunks=8)
            qn0 = load_nat(qd, 0, 512)
            pe_tr(lambda e: xTq[:, e, 0:512], qn0, a_chunks=4, e_chunks=8)
            qproj(0, 0)

            wvn = load_nat(wvd, 0, EL)
            pe_tr(lambda e: wvT[:, e, :], wvn, a_chunks=4, e_chunks=8)
            vT0 = xload_pe(vd, 0, "xv")
            for s_ in range(4):
                vproj(vT0, s_)

            # natural loads for k1/v1 go out now; transposes are fillers
            k1n = load_nat(kd, 512, 512)
            v1n = load_nat(vd, 512, 512)

            # ---------------- filler schedule ----------------------------
            cur = {"kT": None}
            nat_h = {"k1": k1n, "v1": v1n}
            xt_h = {}

            def f_load(key, xd, blk):
                def go():
                    nat_h[key] = load_nat(xd, blk * 512, 512)
                return go

            def f_petr(key, tag, dst=None, tb=None):
                def go():
                    nat = nat_h[key]
                    if dst is None:
                        xt = xtrp.tile(
                            [128, 8, 512], BF16, tag=tag,
                            bufs=(4 if tag == "xk" else 2), name=f"xt_{key}",
                        )
                        pe_tr(lambda e: xt[:, e, :], nat, a_chunks=4, e_chunks=8)
                        xt_h[key] = xt
                    else:
                        pe_tr(
                            lambda e: dst[:, e, tb * 512 : (tb + 1) * 512],
                            nat, a_chunks=4, e_chunks=8,
                        )
                return go

            def f_qtr(key, tb):
                return f_petr(key, "", dst=xTq, tb=tb)

            def f_vproj(key, *ss):
                def go():
                    t = xt_h[key]
                    for s_ in ss:
                        vproj(t, s_)
                return go

            def f_kproj_key(key, c, b):
                def go():
                    kqproj(xt_h[key], wkT, bk_sb, kpT, c, b)
                return go

            def f_qproj(c, tb):
                def go():
                    qproj(c, tb)
                return go

            def f_kproj_fixed(t, c, b):
                def go():
                    kqproj(t, wkT, bk_sb, kpT, c, b)
                return go

            def f_wo():
                def go():
                    won = load_nat(wod, 0, E)
                    dma_tr(woT, 0, won, 8)
                return go

            def f_kload(b):
                def go():
                    cur["kT"] = xload_dmatr(kd, b, "xk")
                return go

            def f_kproj_cur(c, b):
                def go():
                    kqproj(cur["kT"], wkT, bk_sb, kpT, c, b)
                return go

            fillers = {
                (0, 0): {
                    2: [f_petr("k1", "xk"), f_kproj_key("k1", 0, 1),
                        f_load("k2", kd, 2)],
                    4: [f_petr("v1", "xv"), f_vproj("v1", 4, 5),
                        f_load("v2", vd, 2)],
                    5: [f_vproj("v1", 6, 7)],
                    6: [f_petr("k2", "xk"), f_kproj_key("k2", 0, 2),
                        f_load("k3", kd, 3)],
                    8: [f_petr("v2", "xv"), f_vproj("v2", 8, 9),
                        f_load("v3", vd, 3)],
                    9: [f_vproj("v2", 10, 11)],
                    10: [f_petr("k3", "xk"), f_kproj_key("k3", 0, 3),
                         f_load("q1", qd, 1)],
                    12: [f_petr("v3", "xv"), f_vproj("v3", 12, 13),
                         f_load("q2", qd, 2)],
                    13: [f_vproj("v3", 14, 15), f_qtr("q1", 1),
                         f_qproj(0, 1), f_qtr("q2", 2), f_load("q3", qd, 3)],
                },
                (0, 1): {2: [f_qproj(1, 0)], 4: [f_qtr("q3", 3)],
                         13: [f_qproj(0, 2)]},
                (0, 2): {2: [f_qproj(1, 1)], 13: [f_qproj(0, 3)]},
                (0, 3): {2: [f_qproj(1, 2)], 8: [f_qproj(1, 3)]},
                (1, 0): {3: [f_wo()]},
            }
            kT_c0 = None  # c1 projections use xt_h / kT0 below
            # row 0: pair c1 projects from the still-resident prologue k
            # tiles (no re-load); rows 1-2 re-load k blocks (DMA is idle
            # then): load block b at (j, b) slot 7, project at (j, b+1)
            # slot 5 (block 3: same-unit slot 12).
            fillers[(0, 1)].setdefault(5, []).append(f_kproj_fixed(kT0, 1, 0))
            for b in range(1, 4):
                pu, ps_ = ((0, b + 1), 5) if b < 3 else ((0, 3), 12)
                fillers.setdefault(pu, {}).setdefault(ps_, []).append(
                    f_kproj_key(f"k{b}", 1, b)
                )
            for j in range(1, 3):
                c = j + 1
                for b in range(4):
                    fillers.setdefault((j, b), {}).setdefault(7, []).append(
                        f_kload(b)
                    )
                    pu, ps_ = ((j, b + 1), 5) if b < 3 else ((j, 3), 12)
                    fillers.setdefault(pu, {}).setdefault(ps_, []).append(
                        f_kproj_cur(c, b)
                    )
            # q projections for rows 2,3 (pairs c2, c3), JIT one row ahead
            for j in range(1, 3):
                c = j + 1
                for tb in range(4):
                    fillers.setdefault((j, tb), {}).setdefault(10, []).append(
                        f_qproj(c, tb)
                    )

            # ---------------- attention ----------------------------------
            def epilogue(j, tb, ctxA, ctxB):
                tsl = slice(tb * 512, (tb + 1) * 512)
                recf = normp.tile([128, 8], F32, tag="recf", bufs=2)
                nc.vector.reciprocal(
                    out=recf[:, 0:4].rearrange("p (t x) -> p t x", x=1),
                    in_=ctxA.rearrange("p (t x) -> p t x", x=128)[:, :, 64:65],
                )
                nc.vector.reciprocal(
                    out=recf[:, 4:8].rearrange("p (t x) -> p t x", x=1),
                    in_=ctxB.rearrange("p (t x) -> p t x", x=128)[:, :, 64:65],
                )
                ctxn = ctxnp.tile([128, 512], BF16, tag="ctxn", bufs=2)
                for t4 in range(4):
                    nc.vector.tensor_scalar_mul(
                        out=ctxn[:, t4 * 128 : t4 * 128 + 64],
                        in0=ctxA[:, t4 * 128 : t4 * 128 + 64],
                        scalar1=recf[:, t4 : t4 + 1],
                    )
                    nc.vector.tensor_scalar_mul(
                        out=ctxn[:, t4 * 128 + 64 : t4 * 128 + 128],
                        in0=ctxB[:, t4 * 128 : t4 * 128 + 64],
                        scalar1=recf[:, 4 + t4 : 5 + t4],
                    )
                trt = mp_ps.tile([128, 512], BF16, tag="mp")
                for t4 in range(4):
                    nc.tensor.transpose(
                        trt[:, t4 * 128 : (t4 + 1) * 128],
                        ctxn[:, t4 * 128 : (t4 + 1) * 128],
                        ident,
                    )
                nc.vector.tensor_copy(out=ctxnT[:, j, tsl], in_=trt)
                if j == 3:
                    for o in range(8):
                        ps = mp_ps.tile([128, 512], F32, tag="mp")
                        for c in range(4):
                            nc.tensor.matmul(
                                ps,
                                lhsT=woT[:, c, o * 128 : (o + 1) * 128],
                                rhs=ctxnT[:, c, tsl],
                                start=(c == 0),
                                stop=(c == 3),
                            )
                        osb = ctxnp.tile([128, 512], BF16, tag="osb", bufs=2)
                        nc.vector.tensor_copy(out=osb, in_=ps)
                        nc.sync.dma_start(
                            out=outd[o * 128 : (o + 1) * 128, tsl], in_=osb
                        )

            pending_epi = None
            for j in range(4):
                for tb in range(4):
                    tsl = slice(tb * 512, (tb + 1) * 512)
                    hA, hB = 2 * j, 2 * j + 1
                    ctxA = ctx_ps.tile([128, 512], F32, tag="ctx")
                    ctxB = ctx_ps.tile([128, 512], F32, tag="ctx")
                    # one accumulation group per bank: only the FIRST
                    # region's s=0 matmul carries start=True (bank-wide
                    # pending-zero before anything else writes); a start on
                    # every region would re-zero the bank and drop the other
                    # regions' first contribution
                    ufill = fillers.get((j, tb), {})
                    pts = {}

                    def emit_pv(s):
                        for t4 in range(4):
                            nc.tensor.matmul(
                                ctxA[:, t4 * 128 : t4 * 128 + 65],
                                lhsT=pts[s][:, t4 * 128 : (t4 + 1) * 128],
                                rhs=vp_ext[:, s, hA * 65 : hA * 65 + 65],
                                start=(s == 0 and t4 == 0),
                                stop=(s == 15),
                            )
                            nc.tensor.matmul(
                                ctxB[:, t4 * 128 : t4 * 128 + 65],
                                lhsT=pts[s][:, 512 + t4 * 128 : 512 + (t4 + 1) * 128],
                                rhs=vp_ext[:, s, hB * 65 : hB * 65 + 65],
                                start=(s == 0 and t4 == 0),
                                stop=(s == 15),
                            )

                    for s in range(16):
                        ssl = slice(s * 128, (s + 1) * 128)
                        sc = sc_ps.tile([128, 1024], F32, tag="sc")
                        nc.tensor.matmul(
                            sc[:, 0:512],
                            lhsT=kpT[0:64, j, ssl],
                            rhs=qpT[0:64, j, tsl],
                            start=True,
                            stop=True,
                        )
                        nc.tensor.matmul(
                            sc[:, 512:1024],
                            lhsT=kpT[64:128, j, ssl],
                            rhs=qpT[64:128, j, tsl],
                            start=True,
                            stop=True,
                        )
                        pt_t = ptp.tile([128, 1024], BF16, tag="pt", bufs=3)
                        nc.scalar.activation(
                            out=pt_t,
                            in_=sc,
                            func=mybir.ActivationFunctionType.Exp,
                            scale=0.125,
                        )
                        pts[s] = pt_t
                        if s == 2 and pending_epi is not None:
                            pending_epi()
                            pending_epi = None
                        for f in ufill.get(s, []):
                            f()
                        if s >= 2:
                            emit_pv(s - 2)
                    emit_pv(14)
                    emit_pv(15)
                    pending_epi = (
                        lambda j=j, tb=tb, A=ctxA, B=ctxB: epilogue(j, tb, A, B)
                    )
            pending_epi()

    legalize_waits(nc)
    return nc


def _make_in_maps(inputs):
    q, k, v = inputs["q"], inputs["k"], inputs["v"]
    in_maps = []
    for c in range(N_CORES):
        b, hh = c // 2, c % 2
        esl = slice(hh * EL, (hh + 1) * EL)
        in_maps.append(
            {
                "q": np.ascontiguousarray(q[b], dtype=np.float32),
                "k": np.ascontiguousarray(k[b], dtype=np.float32),
                "v": np.ascontiguousarray(v[b], dtype=np.float32),
                "wq": np.ascontiguousarray(inputs["Wq"][esl], dtype=np.float32),
                "wk": np.ascontiguousarray(inputs["Wk"][esl], dtype=np.float32),
                "wv": np.ascontiguousarray(inputs["Wv"][esl], dtype=np.float32),
                "wo": np.ascontiguousarray(inputs["Wo"][:, esl], dtype=np.float32),
                "bq": np.ascontiguousarray(inputs["bq"][esl], dtype=np.float32),
                "bk": np.ascontiguousarray(inputs["bk"][esl], dtype=np.float32),
                "bv": np.ascontiguousarray(inputs["bv"][esl], dtype=np.float32),
            }
        )
    return in_maps


def _gather(results, bo):
    out = np.empty((B, T, E), dtype=np.float32)
    for b in range(B):
        acc = results[2 * b]["outT"].astype(np.float32).T + results[2 * b + 1]["outT"].astype(np.float32).T
        out[b] = acc + bo[None, :]
    return out


def run(inputs, **spmd_kwargs):
    if "nc" not in _CACHED:
        _CACHED["nc"] = build_program()
    nc = _CACHED["nc"]
    in_maps = _make_in_maps(inputs)
    res = run_bass_kernel_spmd(nc, in_maps, core_ids=list(range(N_CORES)), **spmd_kwargs)
    out = _gather(res.results, np.asarray(inputs["bo"], dtype=np.float32))
    return out, res


def kernel(**inputs) -> np.ndarray:
    out, _ = run(inputs)
    return out
